# revision 1
# baseline (speedup 1.0000x reference)
"""CharGPT forward pass on 8 Trainium2 NeuronCores.

Data-parallel over batch: B=8, one batch element per core, no collectives.
Per core: full 6-layer transformer on [T=1024, C=1024] with bf16 matmuls /
f32 accumulation.

Device-side layout conventions:
  - Residual stream x: SBUF [128, 8, 1024] f32  = [t_inner, t_outer, c]
  - LN outputs are PE-transposed to hT [128, 8, 1024] bf16 = [c_in, c_tile, t]
  - q,k projections produced transposed (qT/kT: [c_head, tile, t])
  - attention scores computed transposed S^T [tk, tq] with a ragged causal
    buffer; softmax denominator via ones-matmul over the partition axis;
    normalization folded into the PSUM eviction of att@V.
  - partition-varying biases are applied as [P,1] scalar operands during
    PSUM eviction; free-varying biases (bv, bhead) enter as K=1 matmuls
    against a ones vector. LN gains/shifts are folded into weights/biases
    host-side.
  - attn-out and FFN down-proj computed output-transposed (weights
    stationary, streamed once) then PE-transposed back into the residual.
"""

import os
import sys
from contextlib import ExitStack

if "/opt/trn_rl_repo" not in sys.path:
    sys.path.insert(0, "/opt/trn_rl_repo")

import numpy as np
import ml_dtypes

import concourse.bass as bass
import concourse.tile as tile
from concourse import bacc, mybir
from concourse.bass_utils import run_bass_kernel_spmd

V, C, H, L, T, B = 256, 1024, 16, 6, 1024, 8
HS = C // H          # 64
F = 4 * C            # 4096
EPS = 1e-5
P = 128
NT = T // P          # 8 t-tiles
NCT = C // P         # 8 c-tiles
NF = F // P          # 32 ffn tiles
NV = V // P          # 2 vocab tiles

BF16 = mybir.dt.bfloat16
F32 = mybir.dt.float32
AF = mybir.ActivationFunctionType
ALU = mybir.AluOpType

_BF = ml_dtypes.bfloat16

# ragged causal score buffer: chunk i holds tq in [128*i, T)
_W = [T - P * i for i in range(NT)]            # widths
_OFF = [sum(_W[:i]) for i in range(NT)]        # offsets
_TOT = sum(_W)                                 # 4608

_COMPILED = {}
_DBG = os.environ.get("K_DBG", "")


def _slices(start, end, step=512):
    """[(a,b)] covering [start,end) split on `step` boundaries."""
    out = []
    a = start
    while a < end:
        b = min(end, (a // step + 1) * step)
        out.append((a, b))
        a = b
    return out


def _build_nc():
    nc = bacc.Bacc("TRN2")

    # ---- DRAM I/O ----------------------------------------------------
    ohT_d = nc.dram_tensor("ohT", [P, NV, T], BF16, kind="ExternalInput")
    tokT_d = nc.dram_tensor("tokT", [P, NV, C], BF16, kind="ExternalInput")
    pos_d = nc.dram_tensor("pos", [P, NT, C], F32, kind="ExternalInput")
    wq_d = nc.dram_tensor("wq", [L, P, NCT, C], BF16, kind="ExternalInput")
    wk_d = nc.dram_tensor("wk", [L, P, NCT, C], BF16, kind="ExternalInput")
    wv_d = nc.dram_tensor("wv", [L, P, NCT, C], BF16, kind="ExternalInput")
    wo_d = nc.dram_tensor("wo", [L, P, NCT, C], BF16, kind="ExternalInput")
    w1_d = nc.dram_tensor("w1", [L, P, NCT, F], BF16, kind="ExternalInput")
    w2_d = nc.dram_tensor("w2", [L, P, NF, C], BF16, kind="ExternalInput")
    wh_d = nc.dram_tensor("wh", [P, NCT, V], BF16, kind="ExternalInput")
    bqc_d = nc.dram_tensor("bqc", [L, P, NCT], F32, kind="ExternalInput")
    bkc_d = nc.dram_tensor("bkc", [L, P, NCT], F32, kind="ExternalInput")
    bv_d = nc.dram_tensor("bv", [L, 1, C], F32, kind="ExternalInput")
    boc_d = nc.dram_tensor("boc", [L, P, NCT], F32, kind="ExternalInput")
    b1c_d = nc.dram_tensor("b1c", [L, P, NF], F32, kind="ExternalInput")
    b2c_d = nc.dram_tensor("b2c", [L, P, NCT], F32, kind="ExternalInput")
    bh_d = nc.dram_tensor("bh", [1, V], F32, kind="ExternalInput")
    id_d = nc.dram_tensor("ident", [P, P], BF16, kind="ExternalInput")
    id32_d = nc.dram_tensor("ident32", [P, P], F32, kind="ExternalInput")
    mk_d = nc.dram_tensor("mask", [P, P], BF16, kind="ExternalInput")
    out_d = nc.dram_tensor("out", [P, NT, V], F32, kind="ExternalOutput")

    with tile.TileContext(nc) as tc, ExitStack() as ctx:
        # ---- persistent pools ---------------------------------------
        consts = ctx.enter_context(tc.tile_pool(name="consts", bufs=1))
        xpool = ctx.enter_context(tc.tile_pool(name="xpool", bufs=1))
        wcache = ctx.enter_context(tc.tile_pool(name="wcache", bufs=1))
        wstream = ctx.enter_context(tc.tile_pool(name="wstream", bufs=2))
        biasp = ctx.enter_context(tc.tile_pool(name="biasp", bufs=2))
        small = ctx.enter_context(tc.tile_pool(name="small", bufs=4))
        rrowp = ctx.enter_context(tc.tile_pool(name="rrowp", bufs=2))
        evict = ctx.enter_context(tc.tile_pool(name="evict", bufs=3))
        rpool = ctx.enter_context(tc.tile_pool(name="rpool", bufs=1))
        pp_big = ctx.enter_context(
            tc.tile_pool(name="pp_big", bufs=2, space="PSUM"))
        pp_S = ctx.enter_context(
            tc.tile_pool(name="pp_S", bufs=2, space="PSUM"))
        pp_att = ctx.enter_context(
            tc.tile_pool(name="pp_att", bufs=2, space="PSUM"))
        pp_tr = ctx.enter_context(
            tc.tile_pool(name="pp_tr", bufs=1, space="PSUM"))
        pp_tr32 = ctx.enter_context(
            tc.tile_pool(name="pp_tr32", bufs=1, space="PSUM"))

        ident = consts.tile([P, P], BF16)
        nc.sync.dma_start(ident, id_d[:, :])
        ident32 = consts.tile([P, P], F32)
        nc.sync.dma_start(ident32, id32_d[:, :])
        mask = consts.tile([P, P], BF16)
        nc.sync.dma_start(mask, mk_d[:, :])
        ones_mat = consts.tile([P, P], BF16)
        nc.vector.memset(ones_mat, 1.0)
        eps_t = consts.tile([P, 1], F32)
        nc.vector.memset(eps_t, EPS)

        x_sb = xpool.tile([P, NT, C], F32)

        dbg_state = {"done": False}
        dbg_sb = (xpool.tile([P, NT * V], F32, tag="dbg", name="dbg_sb")
                  if _DBG else None)

        def active():
            return not dbg_state["done"]

        def dbg_write(name, src_ap):
            """If K_DBG==name: cast/copy src (any dtype, [P, <=NT*V] free
            elems) into dbg_sb, DMA to out, and disable later stages."""
            if _DBG != name or dbg_state["done"]:
                return
            n = 1
            for d in src_ap.shape[1:]:
                n *= d
            assert n <= NT * V, n
            nc.vector.tensor_copy(dbg_sb[:, :n], src_ap)
            nc.sync.dma_start(
                out_d[:, :, :],
                dbg_sb.rearrange("p (a b) -> p a b", b=V))
            dbg_state["done"] = True

        def layernorm_transposed(dst_T):
            """x_hat = (x - mean)/sqrt(var+eps) per token; write transpose
            into dst_T ([128, NCT, T] bf16)."""
            for j in range(NT):
                st = small.tile([P, 2, 6], F32, tag="bn_st")
                nc.vector.bn_stats(st[:, 0, :], x_sb[:, j, 0:512])
                nc.vector.bn_stats(st[:, 1, :], x_sb[:, j, 512:1024])
                mv = small.tile([P, 2], F32, tag="bn_mv")
                nc.vector.bn_aggr(mv, st)
                sd = small.tile([P, 1], F32, tag="bn_sd")
                nc.scalar.activation(sd, mv[:, 1:2], AF.Sqrt,
                                     bias=eps_t, scale=1.0)
                rs = small.tile([P, 1], F32, tag="bn_rs")
                nc.vector.reciprocal(rs, sd)
                xh = evict.tile([P, C], BF16, tag="xhat")
                nc.vector.tensor_scalar(
                    xh, x_sb[:, j, :], scalar1=mv[:, 0:1], scalar2=rs,
                    op0=ALU.subtract, op1=ALU.mult,
                )
                for k in range(NCT):
                    ptr = pp_tr.tile([P, P], BF16, tag="tr")
                    nc.tensor.transpose(ptr, xh[:, P * k:P * (k + 1)], ident)
                    nc.vector.tensor_copy(
                        dst_T[:, k, P * j:P * (j + 1)], ptr)

        # ---- embedding: x = onehot @ tok_emb + pos ------------------
        with tc.tile_pool(name="emb", bufs=1) as emb:
            ohT = emb.tile([P, NV, T], BF16)
            nc.sync.dma_start(ohT, ohT_d[:, :, :])
            tokT = emb.tile([P, NV, C], BF16)
            nc.sync.dma_start(tokT, tokT_d[:, :, :])
            pos_sb = emb.tile([P, NT, C], F32)
            nc.sync.dma_start(pos_sb, pos_d[:, :, :])
            for j in range(NT):
                for s in range(2):
                    ps = pp_big.tile([P, 512], F32, tag="big")
                    for vo in range(NV):
                        nc.tensor.matmul(
                            ps, lhsT=ohT[:, vo, P * j:P * (j + 1)],
                            rhs=tokT[:, vo, 512 * s:512 * (s + 1)],
                            start=(vo == 0), stop=(vo == NV - 1),
                        )
                    nc.vector.tensor_add(
                        x_sb[:, j, 512 * s:512 * (s + 1)], ps,
                        pos_sb[:, j, 512 * s:512 * (s + 1)],
                    )
            dbg_write("emb", x_sb[:, 0:2, :])

        # ---- transformer layers -------------------------------------
        for l in range(L):
            if not active():
                break
            # whole-layer weight cache for V projection (DMA early)
            wv_sb = wcache.tile([P, NCT, C], BF16, tag="wv")
            nc.sync.dma_start(wv_sb, wv_d[l])
            # bias columns / rows
            bqc_sb = biasp.tile([P, NCT], F32, tag="bqc")
            nc.sync.dma_start(bqc_sb, bqc_d[l])
            bkc_sb = biasp.tile([P, NCT], F32, tag="bkc")
            nc.sync.dma_start(bkc_sb, bkc_d[l])
            bvb = biasp.tile([P, C], F32, tag="bvb")
            nc.sync.dma_start(bvb, bv_d[l][0:1, :].to_broadcast((P, C)))
            boc_sb = biasp.tile([P, NCT], F32, tag="boc")
            nc.sync.dma_start(boc_sb, boc_d[l])
            b1c_sb = biasp.tile([P, NF], F32, tag="b1c")
            nc.sync.dma_start(b1c_sb, b1c_d[l])
            b2c_sb = biasp.tile([P, NCT], F32, tag="b2c")
            nc.sync.dma_start(b2c_sb, b2c_d[l])

            with tc.tile_pool(name=f"attn{l}", bufs=1) as apool:
                hT = apool.tile([P, NCT, T], BF16, tag="hT")
                layernorm_transposed(hT)
                dbg_write("ln1", hT[:, 0:2, :])

                # ---- q/k projections (transposed outputs) ----------
                qT = apool.tile([P, NCT, T], BF16, tag="qT")
                kT = apool.tile([P, NCT, T], BF16, tag="kT")
                for (w_dram, b_col, dstT, wtag) in (() if not active() else (
                        (wq_d, bqc_sb, qT, "wq"), (wk_d, bkc_sb, kT, "wk"))):
                    for a in range(NCT):
                        wa = wstream.tile([P, NCT, P], BF16, tag=wtag)
                        nc.sync.dma_start(
                            wa, w_dram[l][:, :, P * a:P * (a + 1)])
                        for s in range(2):
                            ps = pp_big.tile([P, 512], F32, tag="big")
                            for k in range(NCT):
                                nc.tensor.matmul(
                                    ps, lhsT=wa[:, k, :],
                                    rhs=hT[:, k, 512 * s:512 * (s + 1)],
                                    start=(k == 0), stop=(k == NCT - 1),
                                )
                            nc.vector.tensor_scalar_add(
                                dstT[:, a, 512 * s:512 * (s + 1)], ps,
                                b_col[:, a:a + 1])

                dbg_write("qt", qT[:, 0:2, :])

                # ---- v projection (normal layout) ------------------
                v_sb = apool.tile([P, NT, C], BF16, tag="v")
                for j in range(NT if active() else 0):
                    for s in range(2):
                        ps = pp_big.tile([P, 512], F32, tag="big")
                        for k in range(NCT):
                            nc.tensor.matmul(
                                ps, lhsT=hT[:, k, P * j:P * (j + 1)],
                                rhs=wv_sb[:, k, 512 * s:512 * (s + 1)],
                                start=(k == 0), stop=(k == NCT - 1),
                            )
                        nc.vector.tensor_add(
                            v_sb[:, j, 512 * s:512 * (s + 1)], ps,
                            bvb[:, 512 * s:512 * (s + 1)])

                dbg_write("v", v_sb[:, 0:2, :])

                # ---- attention, one head-pair at a time ------------
                attTn = apool.tile([P, NCT, T], BF16, tag="hT")
                for m in range(NCT if active() else 0):
                    PT = apool.tile([P, 2, _TOT], BF16, tag="PT")
                    Rm = rpool.tile([P, T], F32, tag="R")
                    for h2 in range(2):
                        hb = 64 * h2
                        # scores S^T = k^T q (per tk chunk), exp, mask
                        for i in range(NT):
                            n0 = P * i
                            for (c0, c1) in _slices(n0, T):
                                ps = pp_S.tile([P, 512], F32, tag="S")
                                nc.tensor.matmul(
                                    ps[:, :c1 - c0],
                                    lhsT=kT[hb:hb + 64, m, P * i:P * (i + 1)],
                                    rhs=qT[hb:hb + 64, m, c0:c1],
                                    start=True, stop=True,
                                )
                                f0 = _OFF[i] + c0 - n0
                                nc.scalar.activation(
                                    PT[:, h2, f0:f0 + c1 - c0],
                                    ps[:, :c1 - c0], AF.Exp, scale=0.125)
                            nc.vector.tensor_mul(
                                PT[:, h2, _OFF[i]:_OFF[i] + P],
                                PT[:, h2, _OFF[i]:_OFF[i] + P], mask)
                        # denominator: ones-matrix matmul puts the column
                        # sum on every output partition; reciprocal lands
                        # partition-aligned into Rm — no broadcast needed.
                        for s in range(2):
                            dn = pp_S.tile([P, 512], F32, tag="S")
                            lo, hi = 512 * s, 512 * (s + 1)
                            idxs = [i for i in range(NT) if P * i < hi]
                            for i in idxs:
                                c0 = max(lo, P * i)
                                f0 = _OFF[i] + c0 - P * i
                                nc.tensor.matmul(
                                    dn[:, c0 - lo:hi - lo],
                                    lhsT=ones_mat[:, :],
                                    rhs=PT[:, h2, f0:f0 + hi - c0],
                                    start=(i == idxs[0]),
                                    stop=(i == idxs[-1]),
                                )
                            nc.vector.reciprocal(
                                Rm[hb:hb + 64, lo:hi],
                                dn[hb:hb + 64, 0:hi - lo])
                    if m == 0:
                        dbg_write("pt", PT[:, 0, 0:NT * V])
                        dbg_write("rm", Rm[:, :])
                    # att = V-weighted P^T, normalized on eviction.
                    # one PSUM bank per head: start=True clears the whole
                    # bank, so head groups must not share one.
                    for j in range(NT if active() else 0):
                        pa0 = pp_att.tile([P, P], F32, tag="att")
                        pa1 = pp_att.tile([P, P], F32, tag="att")
                        pas = (pa0, pa1)
                        for i in range(j + 1):
                            f0 = _OFF[i] + P * (j - i)
                            for h2 in range(2):
                                nc.tensor.matmul(
                                    pas[h2][64 * h2:64 * (h2 + 1), :],
                                    lhsT=v_sb[:, i, P * m + 64 * h2:
                                              P * m + 64 * (h2 + 1)],
                                    rhs=PT[:, h2, f0:f0 + P],
                                    start=(i == 0), stop=(i == j),
                                    tile_position=(0, 64 * h2),
                                )
                        for h2 in range(2):
                            hb = 64 * h2
                            nc.vector.tensor_mul(
                                attTn[hb:hb + 64, m, P * j:P * (j + 1)],
                                pas[h2][hb:hb + 64, :],
                                Rm[hb:hb + 64, P * j:P * (j + 1)])
                    if m == 0:
                        dbg_write("att", attTn[:, 0, :])

                # ---- output projection (transposed) + residual -----
                for a in range(NCT if active() else 0):
                    woa = wstream.tile([P, NCT, P], BF16, tag="wo")
                    nc.sync.dma_start(woa, wo_d[l][:, :, P * a:P * (a + 1)])
                    for s in range(2):
                        ps = pp_big.tile([P, 512], F32, tag="big")
                        for ct in range(NCT):
                            nc.tensor.matmul(
                                ps, lhsT=woa[:, ct, :],
                                rhs=attTn[:, ct, 512 * s:512 * (s + 1)],
                                start=(ct == 0), stop=(ct == NCT - 1),
                            )
                        oT = evict.tile([P, 512], F32, tag="oT")
                        nc.vector.tensor_scalar_add(oT, ps, boc_sb[:, a:a + 1])
                        for tb in range(4):
                            ptr = pp_tr32.tile([P, P], F32, tag="tr32")
                            nc.tensor.transpose(
                                ptr, oT[:, P * tb:P * (tb + 1)], ident32)
                            jj = 4 * s + tb
                            nc.vector.tensor_add(
                                x_sb[:, jj, P * a:P * (a + 1)],
                                x_sb[:, jj, P * a:P * (a + 1)], ptr)

            if l == 0:
                dbg_write("attnout", x_sb[:, 0:2, :])

            # ---- FFN ------------------------------------------------
            with tc.tile_pool(name=f"ffn{l}", bufs=1) as fpool:
                h2T = fpool.tile([P, NCT, T], BF16, tag="h2T")
                if active():
                    layernorm_transposed(h2T)

                for ht in range(2 if active() else 0):
                    t0 = 512 * ht
                    zT = fpool.tile([P, NF, 512], BF16, tag="zT")
                    for u in range(NF):
                        w1u = wstream.tile([P, NCT, P], BF16, tag="w1")
                        nc.sync.dma_start(
                            w1u, w1_d[l][:, :, P * u:P * (u + 1)])
                        ps = pp_big.tile([P, 512], F32, tag="big")
                        for k in range(NCT):
                            nc.tensor.matmul(
                                ps, lhsT=w1u[:, k, :],
                                rhs=h2T[:, k, t0:t0 + 512],
                                start=(k == 0), stop=(k == NCT - 1),
                            )
                        nc.scalar.activation(zT[:, u, :], ps, AF.Relu,
                                             bias=b1c_sb[:, u:u + 1],
                                             scale=1.0)
                    # down-projection: W2 stationary → transposed output,
                    # transpose back into the residual
                    for a in range(NCT):
                        ps = pp_big.tile([P, 512], F32, tag="big")
                        for uh in range(2):
                            w2a = wstream.tile([P, 16, P], BF16, tag="w2")
                            nc.sync.dma_start(
                                w2a,
                                w2_d[l][:, 16 * uh:16 * (uh + 1),
                                        P * a:P * (a + 1)])
                            for u16 in range(16):
                                u = 16 * uh + u16
                                nc.tensor.matmul(
                                    ps, lhsT=w2a[:, u16, :], rhs=zT[:, u, :],
                                    start=(u == 0), stop=(u == NF - 1),
                                )
                        oT = evict.tile([P, 512], F32, tag="oT")
                        nc.vector.tensor_scalar_add(oT, ps, b2c_sb[:, a:a + 1])
                        for tb in range(4):
                            ptr = pp_tr32.tile([P, P], F32, tag="tr32")
                            nc.tensor.transpose(
                                ptr, oT[:, P * tb:P * (tb + 1)], ident32)
                            jj = 4 * ht + tb
                            nc.vector.tensor_add(
                                x_sb[:, jj, P * a:P * (a + 1)],
                                x_sb[:, jj, P * a:P * (a + 1)], ptr)
            if l == 0:
                dbg_write("ffn", x_sb[:, 0:2, :])

        # ---- final layernorm + head ---------------------------------
        with tc.tile_pool(name="head", bufs=1) as hpool:
          if active():
            xfT = hpool.tile([P, NCT, T], BF16, tag="xfT")
            layernorm_transposed(xfT)
            wh_sb = hpool.tile([P, NCT, V], BF16, tag="wh")
            nc.sync.dma_start(wh_sb, wh_d[:, :, :])
            bhb = hpool.tile([P, V], F32, tag="bhb")
            nc.sync.dma_start(bhb, bh_d[0:1, :].to_broadcast((P, V)))
            out_sb = hpool.tile([P, NT, V], F32, tag="out")
            for j in range(NT):
                ps = pp_S.tile([P, 512], F32, tag="S")
                for k in range(NCT):
                    nc.tensor.matmul(
                        ps[:, :V], lhsT=xfT[:, k, P * j:P * (j + 1)],
                        rhs=wh_sb[:, k, :],
                        start=(k == 0), stop=(k == NCT - 1),
                    )
                nc.vector.tensor_add(out_sb[:, j, :], ps[:, :V], bhb)
            nc.sync.dma_start(out_d[:, :, :], out_sb)

    nc.finalize()
    return nc


def _prep_inputs(inputs):
    """Host-side preprocessing: fold LN gains/shifts into weights and
    effective bias columns/rows, rearrange to device layouts, cast bf16."""
    f = {k: np.asarray(v) for k, v in inputs.items()}

    def t8(w, ko):  # [(ko*128), n] -> [128, ko, n]
        n = w.shape[1]
        return np.ascontiguousarray(w.reshape(ko, P, n).transpose(1, 0, 2))

    def col(b, ko):  # [ko*128] -> [128, ko]
        return np.ascontiguousarray(b.reshape(ko, P).T).astype(np.float32)

    g1 = f["ln1_g"][:, :, None]  # [L, C, 1]
    b1n = f["ln1_b"]
    g2 = f["ln2_g"][:, :, None]
    b2n = f["ln2_b"]

    wq = np.stack([t8(f["Wq"][l] * g1[l], NCT) for l in range(L)])
    wk = np.stack([t8(f["Wk"][l] * g1[l], NCT) for l in range(L)])
    wv = np.stack([t8(f["Wv"][l] * g1[l], NCT) for l in range(L)])
    wo = np.stack([t8(f["Wo"][l], NCT) for l in range(L)])
    w1 = np.stack([t8(f["W1"][l] * g2[l], NCT) for l in range(L)])
    w2 = np.stack([t8(f["W2"][l], NF) for l in range(L)])
    wh = t8(f["Whead"] * f["lnf_g"][:, None], NCT)

    bqc = np.stack([col(b1n[l] @ f["Wq"][l], NCT) for l in range(L)])
    bkc = np.stack([col(b1n[l] @ f["Wk"][l], NCT) for l in range(L)])
    bv = np.stack([(b1n[l] @ f["Wv"][l])[None] for l in range(L)])
    boc = np.stack([col(f["bo"][l], NCT) for l in range(L)])
    b1c = np.stack([col(b2n[l] @ f["W1"][l] + f["b1"][l], NF)
                    for l in range(L)])
    b2c = np.stack([col(f["b2"][l], NCT) for l in range(L)])
    bh = (f["lnf_b"] @ (f["Whead"] * f["lnf_g"][:, None]) + f["bhead"])[None]

    tokT = t8(f["tok_emb"], NV)
    pos = t8(f["pos_emb"][:T], NT).astype(np.float32)

    common = {
        "wq": wq.astype(_BF), "wk": wk.astype(_BF), "wv": wv.astype(_BF),
        "wo": wo.astype(_BF), "w1": w1.astype(_BF), "w2": w2.astype(_BF),
        "wh": wh.astype(_BF),
        "bqc": bqc, "bkc": bkc, "boc": boc, "b1c": b1c, "b2c": b2c,
        "bv": bv.astype(np.float32), "bh": bh.astype(np.float32),
        "tokT": tokT.astype(_BF), "pos": pos,
        "ident": np.eye(P, dtype=_BF),
        "ident32": np.eye(P, dtype=np.float32),
        "mask": np.triu(np.ones((P, P))).astype(_BF),
    }

    idx = f["idx"].astype(np.int64)
    in_maps = []
    for b in range(B):
        oh = (np.arange(V)[:, None] == idx[b][None, :]).astype(np.float32)
        ohT = np.ascontiguousarray(
            oh.reshape(NV, P, T).transpose(1, 0, 2)).astype(_BF)
        m = dict(common)
        m["ohT"] = ohT
        in_maps.append(m)
    return in_maps


def kernel(**inputs):
    if "nc" not in _COMPILED:
        _COMPILED["nc"] = _build_nc()
    nc = _COMPILED["nc"]
    in_maps = _prep_inputs(inputs)
    res = run_bass_kernel_spmd(nc, in_maps, core_ids=list(range(B)))
    outs = []
    for b in range(B):
        o = np.asarray(res.results[b]["out"])  # [128, 8, 256]
        outs.append(o.transpose(1, 0, 2).reshape(T, V))
    return np.stack(outs).astype(np.float32)



# revision 9
# speedup vs baseline: 1.3132x; 1.3132x over previous
"""CharGPT forward pass on 8 Trainium2 NeuronCores.

Data-parallel over batch: B=8, one batch element per core, no collectives.
Per core: full 6-layer transformer on [T=1024, C=1024] with bf16 matmuls /
f32 accumulation.

Device-side layout conventions:
  - Residual stream x: SBUF [128, 8, 1024] f32  = [t_inner, t_outer, c]
  - LN outputs are PE-transposed to hT [128, 8, 1024] bf16 = [c_in, c_tile, t]
  - q,k projections produced transposed (qT/kT: [c_head, tile, t]) from
    per-a streamed weight tiles (contiguous 256KB DMAs).
  - attention scores computed transposed S^T [tk, tq] with a ragged causal
    buffer; softmax denominator via ones-matmul over the partition axis;
    normalization folded into the PSUM eviction of att@V (both heads of a
    pair accumulate into ONE psum bank via col-group tile_position).
  - attn-out and FFN down-proj computed directly in residual layout [t, c]
    (activation tiles stationary, cached/streamed weights moving), so no
    PE transposes on the output path; row biases (bo, b2, bv) enter the
    PSUM accumulation as K=1 matmuls against a ones row.
  - Wv/Wo cached whole-layer (2MB contiguous DMAs); Wq/Wk/W1 streamed as
    contiguous pre-tiled 256KB tiles; W2 streamed as contiguous 2MB
    c-quarters. LN gains/shifts folded into weights/biases host-side.
"""

import os
import sys
from contextlib import ExitStack

if "/opt/trn_rl_repo" not in sys.path:
    sys.path.insert(0, "/opt/trn_rl_repo")

import numpy as np
import ml_dtypes

import concourse.bass as bass
import concourse.tile as tile
from concourse import bacc, mybir
from concourse.bass_utils import run_bass_kernel_spmd

V, C, H, L, T, B = 256, 1024, 16, 6, 1024, 8
HS = C // H          # 64
F = 4 * C            # 4096
EPS = 1e-5
P = 128
NT = T // P          # 8 t-tiles
NCT = C // P         # 8 c-tiles
NF = F // P          # 32 ffn tiles
NV = V // P          # 2 vocab tiles

BF16 = mybir.dt.bfloat16
F32 = mybir.dt.float32
AF = mybir.ActivationFunctionType
ALU = mybir.AluOpType

_BF = ml_dtypes.bfloat16

# ragged causal score buffer: chunk i holds tq in [128*i, T)
_W = [T - P * i for i in range(NT)]            # widths
_OFF = [sum(_W[:i]) for i in range(NT)]        # offsets
_TOT = sum(_W)                                 # 4608

_COMPILED = {}
_DBG = os.environ.get("K_DBG", "")


def _slices(start, end, step=512):
    """[(a,b)] covering [start,end) split on `step` boundaries."""
    out = []
    a = start
    while a < end:
        b = min(end, (a // step + 1) * step)
        out.append((a, b))
        a = b
    return out


def _build_nc():
    nc = bacc.Bacc("TRN2")

    # ---- DRAM I/O ----------------------------------------------------
    ohT_d = nc.dram_tensor("ohT", [P, NV, T], BF16, kind="ExternalInput")
    tokT_d = nc.dram_tensor("tokT", [P, NV, C], BF16, kind="ExternalInput")
    pos_d = nc.dram_tensor("pos", [P, NT, C], F32, kind="ExternalInput")
    # wq/wk pre-tiled a-major: [L, a, p, k, j]
    wq_d = nc.dram_tensor("wq", [L, NCT, P, NCT, P], BF16,
                          kind="ExternalInput")
    wk_d = nc.dram_tensor("wk", [L, NCT, P, NCT, P], BF16,
                          kind="ExternalInput")
    wv_d = nc.dram_tensor("wv", [L, P, NCT, C], BF16, kind="ExternalInput")
    wo_d = nc.dram_tensor("wo", [L, P, NCT, C], BF16, kind="ExternalInput")
    # w1 pre-tiled u-major: [L, u, p, k, j]
    w1_d = nc.dram_tensor("w1", [L, NF, P, NCT, P], BF16,
                          kind="ExternalInput")
    # w2 pre-tiled c-quarter-major: [L, q, p, u, c]
    w2_d = nc.dram_tensor("w2", [L, 4, P, NF, 256], BF16,
                          kind="ExternalInput")
    wh_d = nc.dram_tensor("wh", [P, NCT, V], BF16, kind="ExternalInput")
    bqc_d = nc.dram_tensor("bqc", [L, P, NCT], F32, kind="ExternalInput")
    bkc_d = nc.dram_tensor("bkc", [L, P, NCT], F32, kind="ExternalInput")
    # free-varying bias rows (K=1 matmul operands): bv, bo, b2
    bvr_d = nc.dram_tensor("bvr", [L, 1, C], BF16, kind="ExternalInput")
    bor_d = nc.dram_tensor("bor", [L, 1, C], BF16, kind="ExternalInput")
    b2r_d = nc.dram_tensor("b2r", [L, 1, C], BF16, kind="ExternalInput")
    b1c_d = nc.dram_tensor("b1c", [L, P, NF], F32, kind="ExternalInput")
    bh_d = nc.dram_tensor("bh", [1, V], F32, kind="ExternalInput")
    id_d = nc.dram_tensor("ident", [P, P], BF16, kind="ExternalInput")
    mk_d = nc.dram_tensor("mask", [P, P], BF16, kind="ExternalInput")
    out_d = nc.dram_tensor("out", [P, NT, V], F32, kind="ExternalOutput")

    with tile.TileContext(nc) as tc, ExitStack() as ctx:
        # ---- persistent pools ---------------------------------------
        consts = ctx.enter_context(tc.tile_pool(name="consts", bufs=1))
        xpool = ctx.enter_context(tc.tile_pool(name="xpool", bufs=1))
        biasp = ctx.enter_context(tc.tile_pool(name="biasp", bufs=1))
        small = ctx.enter_context(tc.tile_pool(name="small", bufs=4))
        evict = ctx.enter_context(tc.tile_pool(name="evict", bufs=3))
        pp_big = ctx.enter_context(
            tc.tile_pool(name="pp_big", bufs=3, space="PSUM"))
        pp_S = ctx.enter_context(
            tc.tile_pool(name="pp_S", bufs=3, space="PSUM"))
        pp_attr = ctx.enter_context(
            tc.tile_pool(name="pp_attr", bufs=2, space="PSUM"))

        ident = consts.tile([P, P], BF16)
        nc.sync.dma_start(ident, id_d[:, :])
        mask = consts.tile([P, P], BF16)
        nc.sync.dma_start(mask, mk_d[:, :])
        ones_mat = consts.tile([P, P], BF16)
        nc.vector.memset(ones_mat, 1.0)
        eps_t = consts.tile([P, 1], F32)
        nc.vector.memset(eps_t, EPS)

        x_sb = xpool.tile([P, NT, C], F32)

        dbg_state = {"done": False}
        dbg_sb = (xpool.tile([P, NT * V], F32, tag="dbg", name="dbg_sb")
                  if _DBG else None)

        def active():
            return not dbg_state["done"]

        def dbg_write(name, src_ap):
            """If K_DBG==name: cast/copy src (any dtype, [P, <=NT*V] free
            elems) into dbg_sb, DMA to out, and disable later stages."""
            if _DBG != name or dbg_state["done"]:
                return
            n = 1
            for d in src_ap.shape[1:]:
                n *= d
            assert n <= NT * V, n
            nc.vector.tensor_copy(dbg_sb[:, :n], src_ap)
            nc.sync.dma_start(
                out_d[:, :, :],
                dbg_sb.rearrange("p (a b) -> p a b", b=V))
            dbg_state["done"] = True

        def layernorm_transposed(dst_T):
            """x_hat = (x - mean)/sqrt(var+eps) per token; write transpose
            into dst_T ([128, NCT, T] bf16)."""
            for j in range(NT):
                st = small.tile([P, 2, 6], F32, tag="bn_st")
                nc.vector.bn_stats(st[:, 0, :], x_sb[:, j, 0:512])
                nc.vector.bn_stats(st[:, 1, :], x_sb[:, j, 512:1024])
                mv = small.tile([P, 2], F32, tag="bn_mv")
                nc.vector.bn_aggr(mv, st)
                sd = small.tile([P, 1], F32, tag="bn_sd")
                nc.scalar.activation(sd, mv[:, 1:2], AF.Sqrt,
                                     bias=eps_t, scale=1.0)
                rs = small.tile([P, 1], F32, tag="bn_rs")
                nc.vector.reciprocal(rs, sd)
                xh = evict.tile([P, C], BF16, tag="xhat")
                nc.vector.tensor_scalar(
                    xh, x_sb[:, j, :], scalar1=mv[:, 0:1], scalar2=rs,
                    op0=ALU.subtract, op1=ALU.mult,
                )
                for kb in range(2):
                    ptr = pp_attr.tile([P, 4, P], BF16, tag="att")
                    for k4 in range(4):
                        k = 4 * kb + k4
                        nc.tensor.transpose(
                            ptr[:, k4, :], xh[:, P * k:P * (k + 1)], ident)
                    nc.vector.tensor_copy(
                        dst_T[:, 4 * kb:4 * kb + 4, P * j:P * (j + 1)], ptr)

        # ---- embedding: x = onehot @ tok_emb + pos ------------------
        with tc.tile_pool(name="emb", bufs=1) as emb:
            ohT = emb.tile([P, NV, T], BF16)
            nc.sync.dma_start(ohT, ohT_d[:, :, :])
            tokT = emb.tile([P, NV, C], BF16)
            nc.sync.dma_start(tokT, tokT_d[:, :, :])
            pos_sb = emb.tile([P, NT, C], F32)
            nc.sync.dma_start(pos_sb, pos_d[:, :, :])
            for j in range(NT):
                for s in range(2):
                    ps = pp_big.tile([P, 512], F32, tag="big")
                    for vo in range(NV):
                        nc.tensor.matmul(
                            ps, lhsT=ohT[:, vo, P * j:P * (j + 1)],
                            rhs=tokT[:, vo, 512 * s:512 * (s + 1)],
                            start=(vo == 0), stop=(vo == NV - 1),
                        )
                    nc.vector.tensor_add(
                        x_sb[:, j, 512 * s:512 * (s + 1)], ps,
                        pos_sb[:, j, 512 * s:512 * (s + 1)],
                    )
            dbg_write("emb", x_sb[:, 0:2, :])

        # ---- transformer layers -------------------------------------
        for l in range(L):
            if not active():
                break
            # bias columns / rows
            bqc_sb = biasp.tile([P, NCT], F32, tag="bqc")
            nc.sync.dma_start(bqc_sb, bqc_d[l])
            bkc_sb = biasp.tile([P, NCT], F32, tag="bkc")
            nc.sync.dma_start(bkc_sb, bkc_d[l])
            bvr = biasp.tile([1, C], BF16, tag="bvr")
            nc.sync.dma_start(bvr, bvr_d[l])
            bor = biasp.tile([1, C], BF16, tag="bor")
            nc.sync.dma_start(bor, bor_d[l])
            b2r = biasp.tile([1, C], BF16, tag="b2r")
            nc.sync.dma_start(b2r, b2r_d[l])
            b1c_sb = biasp.tile([P, NF], F32, tag="b1c")
            nc.sync.dma_start(b1c_sb, b1c_d[l])

            with tc.tile_pool(name=f"attn{l}", bufs=1) as apool, \
                    tc.tile_pool(name=f"qks{l}", bufs=2) as qkstream:
                # whole-layer caches for Wv/Wo (contiguous 2MB DMAs)
                wv_sb = apool.tile([P, NCT, C], BF16, tag="wv")
                nc.sync.dma_start(wv_sb, wv_d[l])
                wo_sb = apool.tile([P, NCT, C], BF16, tag="wo")
                nc.sync.dma_start(wo_sb, wo_d[l])
                hT = apool.tile([P, NCT, T], BF16, tag="hT")
                layernorm_transposed(hT)
                dbg_write("ln1", hT[:, 0:2, :])

                # ---- q/k projections (transposed outputs) ----------
                qT = apool.tile([P, NCT, T], BF16, tag="qT")
                kT = apool.tile([P, NCT, T], BF16, tag="kT")
                for (w_dram, b_col, dstT, wtag) in (() if not active() else (
                        (wq_d, bqc_sb, qT, "wq"), (wk_d, bkc_sb, kT, "wk"))):
                    for a in range(NCT):
                        wa = qkstream.tile([P, NCT, P], BF16, tag=wtag)
                        nc.sync.dma_start(wa, w_dram[l, a])
                        for s in range(2):
                            ps = pp_big.tile([P, 512], F32, tag="big")
                            for k in range(NCT):
                                nc.tensor.matmul(
                                    ps, lhsT=wa[:, k, :],
                                    rhs=hT[:, k, 512 * s:512 * (s + 1)],
                                    start=(k == 0), stop=(k == NCT - 1),
                                )
                            nc.vector.tensor_scalar_add(
                                dstT[:, a, 512 * s:512 * (s + 1)], ps,
                                b_col[:, a:a + 1])

                dbg_write("qt", qT[:, 0:2, :])

                # ---- v projection (normal layout, bias via K=1 mm) --
                v_sb = apool.tile([P, NT, C], BF16, tag="v")
                for j in range(NT if active() else 0):
                    for s in range(2):
                        ps = pp_big.tile([P, 512], F32, tag="big")
                        nc.tensor.matmul(
                            ps, lhsT=ones_mat[0:1, :],
                            rhs=bvr[0:1, 512 * s:512 * (s + 1)],
                            start=True, stop=False,
                        )
                        for k in range(NCT):
                            nc.tensor.matmul(
                                ps, lhsT=hT[:, k, P * j:P * (j + 1)],
                                rhs=wv_sb[:, k, 512 * s:512 * (s + 1)],
                                start=False, stop=(k == NCT - 1),
                            )
                        nc.vector.tensor_copy(
                            v_sb[:, j, 512 * s:512 * (s + 1)], ps)

                dbg_write("v", v_sb[:, 0:2, :])

                # ---- attention, one head-pair at a time ------------
                attTn = apool.tile([P, NCT, T], BF16, tag="hT")
                for m in range(NCT if active() else 0):
                    PT = apool.tile([P, 2, _TOT], BF16, tag=f"PT{m % 2}")
                    Rm = apool.tile([P, T], F32, tag="R")
                    for h2 in range(2):
                        hb = 64 * h2
                        # scores S^T = k^T q (per tk chunk), exp, mask
                        for i in range(NT):
                            n0 = P * i
                            for (c0, c1) in _slices(n0, T):
                                ps = pp_S.tile([P, 512], F32, tag="S")
                                nc.tensor.matmul(
                                    ps[:, :c1 - c0],
                                    lhsT=kT[hb:hb + 64, m, P * i:P * (i + 1)],
                                    rhs=qT[hb:hb + 64, m, c0:c1],
                                    start=True, stop=True,
                                )
                                f0 = _OFF[i] + c0 - n0
                                nc.scalar.activation(
                                    PT[:, h2, f0:f0 + c1 - c0],
                                    ps[:, :c1 - c0], AF.Exp, scale=0.125)
                            nc.vector.tensor_mul(
                                PT[:, h2, _OFF[i]:_OFF[i] + P],
                                PT[:, h2, _OFF[i]:_OFF[i] + P], mask)
                        # denominator: ones-matrix matmul puts the column
                        # sum on every output partition; reciprocal lands
                        # partition-aligned into Rm — no broadcast needed.
                        for s in range(2):
                            dn = pp_S.tile([P, 512], F32, tag="S")
                            lo, hi = 512 * s, 512 * (s + 1)
                            idxs = [i for i in range(NT) if P * i < hi]
                            for i in idxs:
                                c0 = max(lo, P * i)
                                f0 = _OFF[i] + c0 - P * i
                                nc.tensor.matmul(
                                    dn[:, c0 - lo:hi - lo],
                                    lhsT=ones_mat[:, :],
                                    rhs=PT[:, h2, f0:f0 + hi - c0],
                                    start=(i == idxs[0]),
                                    stop=(i == idxs[-1]),
                                )
                            nc.vector.reciprocal(
                                Rm[hb:hb + 64, lo:hi],
                                dn[hb:hb + 64, 0:hi - lo])
                    if m == 0:
                        dbg_write("pt", PT[:, 0, 0:NT * V])
                        dbg_write("rm", Rm[:, :])
                    # att = V-weighted P^T, normalized on eviction.
                    # one PSUM bank per head: start=True clears the whole
                    # bank, so head groups must not share one.
                    for j in range(NT if active() else 0):
                        pa0 = pp_attr.tile([P, P], F32, tag="att")
                        pa1 = pp_attr.tile([P, P], F32, tag="att")
                        pas = (pa0, pa1)
                        for i in range(j + 1):
                            f0 = _OFF[i] + P * (j - i)
                            for h2 in range(2):
                                nc.tensor.matmul(
                                    pas[h2][64 * h2:64 * (h2 + 1), :],
                                    lhsT=v_sb[:, i, P * m + 64 * h2:
                                              P * m + 64 * (h2 + 1)],
                                    rhs=PT[:, h2, f0:f0 + P],
                                    start=(i == 0), stop=(i == j),
                                    tile_position=(0, 64 * h2),
                                )
                        for h2 in range(2):
                            hb = 64 * h2
                            nc.vector.tensor_mul(
                                attTn[hb:hb + 64, m, P * j:P * (j + 1)],
                                pas[h2][hb:hb + 64, :],
                                Rm[hb:hb + 64, P * j:P * (j + 1)])
                    if m == 0:
                        dbg_write("att", attTn[:, 0, :])

                # ---- output projection direct into residual --------
                for j in range(NT if active() else 0):
                    for s in range(2):
                        ps = pp_big.tile([P, 512], F32, tag="big")
                        nc.tensor.matmul(
                            ps, lhsT=ones_mat[0:1, :],
                            rhs=bor[0:1, 512 * s:512 * (s + 1)],
                            start=True, stop=False,
                        )
                        for ct in range(NCT):
                            nc.tensor.matmul(
                                ps, lhsT=attTn[:, ct, P * j:P * (j + 1)],
                                rhs=wo_sb[:, ct, 512 * s:512 * (s + 1)],
                                start=False, stop=(ct == NCT - 1),
                            )
                        nc.vector.tensor_add(
                            x_sb[:, j, 512 * s:512 * (s + 1)],
                            x_sb[:, j, 512 * s:512 * (s + 1)], ps)

            if l == 0:
                dbg_write("attnout", x_sb[:, 0:2, :])

            # ---- FFN ------------------------------------------------
            with tc.tile_pool(name=f"ffn{l}", bufs=1) as fpool, \
                    tc.tile_pool(name=f"w1s{l}", bufs=3) as w1stream, \
                    tc.tile_pool(name=f"w2s{l}", bufs=2) as w2pool:
                h2T = fpool.tile([P, NCT, T], BF16, tag="h2T")
                if active():
                    layernorm_transposed(h2T)

                zT = fpool.tile([P, NF, T], BF16, tag="zT")
                for u in range(NF if active() else 0):
                    w1u = w1stream.tile([P, NCT, P], BF16, tag="w1")
                    nc.sync.dma_start(w1u, w1_d[l, u])
                    for s in range(2):
                        ps = pp_big.tile([P, 512], F32, tag="big")
                        for k in range(NCT):
                            nc.tensor.matmul(
                                ps, lhsT=w1u[:, k, :],
                                rhs=h2T[:, k, 512 * s:512 * (s + 1)],
                                start=(k == 0), stop=(k == NCT - 1),
                            )
                        nc.scalar.activation(
                            zT[:, u, 512 * s:512 * (s + 1)], ps, AF.Relu,
                            bias=b1c_sb[:, u:u + 1], scale=1.0)
                # down-projection direct into residual, c-quarters
                for q in range(4 if active() else 0):
                    w2q = w2pool.tile([P, NF, 256], BF16, tag="w2")
                    nc.sync.dma_start(w2q, w2_d[l, q])
                    for j in range(NT):
                        ps = pp_big.tile([P, 512], F32, tag="big")
                        nc.tensor.matmul(
                            ps[:, :256], lhsT=ones_mat[0:1, :],
                            rhs=b2r[0:1, 256 * q:256 * (q + 1)],
                            start=True, stop=False,
                        )
                        for u in range(NF):
                            nc.tensor.matmul(
                                ps[:, :256],
                                lhsT=zT[:, u, P * j:P * (j + 1)],
                                rhs=w2q[:, u, :],
                                start=False, stop=(u == NF - 1),
                            )
                        nc.vector.tensor_add(
                            x_sb[:, j, 256 * q:256 * (q + 1)],
                            x_sb[:, j, 256 * q:256 * (q + 1)],
                            ps[:, :256])
            if l == 0:
                dbg_write("ffn", x_sb[:, 0:2, :])

        # ---- final layernorm + head ---------------------------------
        with tc.tile_pool(name="head", bufs=1) as hpool:
          if active():
            xfT = hpool.tile([P, NCT, T], BF16, tag="xfT")
            layernorm_transposed(xfT)
            wh_sb = hpool.tile([P, NCT, V], BF16, tag="wh")
            nc.sync.dma_start(wh_sb, wh_d[:, :, :])
            bhb = hpool.tile([P, V], F32, tag="bhb")
            nc.sync.dma_start(bhb, bh_d[0:1, :].to_broadcast((P, V)))
            out_sb = hpool.tile([P, NT, V], F32, tag="out")
            for j in range(NT):
                ps = pp_big.tile([P, 512], F32, tag="big")
                for k in range(NCT):
                    nc.tensor.matmul(
                        ps[:, :V], lhsT=xfT[:, k, P * j:P * (j + 1)],
                        rhs=wh_sb[:, k, :],
                        start=(k == 0), stop=(k == NCT - 1),
                    )
                nc.vector.tensor_add(out_sb[:, j, :], ps[:, :V], bhb)
            nc.sync.dma_start(out_d[:, :, :], out_sb)

    nc.finalize()
    return nc


def _prep_inputs(inputs):
    """Host-side preprocessing: fold LN gains/shifts into weights and
    effective bias columns/rows, rearrange to device layouts, cast bf16."""
    f = {k: np.asarray(v) for k, v in inputs.items()}

    def t8(w, ko):  # [(ko*128), n] -> [128, ko, n]
        n = w.shape[1]
        return np.ascontiguousarray(w.reshape(ko, P, n).transpose(1, 0, 2))

    def tile_out(w, no):  # [C, (no*128)] -> [no, 128, ki, 128] a-major
        ki = w.shape[0] // P
        return np.ascontiguousarray(
            w.reshape(ki, P, no, P).transpose(2, 1, 0, 3))

    def col(b, ko):  # [ko*128] -> [128, ko]
        return np.ascontiguousarray(b.reshape(ko, P).T).astype(np.float32)

    g1 = f["ln1_g"][:, :, None]  # [L, C, 1]
    b1n = f["ln1_b"]
    g2 = f["ln2_g"][:, :, None]
    b2n = f["ln2_b"]

    wq = np.stack([tile_out(f["Wq"][l] * g1[l], NCT) for l in range(L)])
    wk = np.stack([tile_out(f["Wk"][l] * g1[l], NCT) for l in range(L)])
    wv = np.stack([t8(f["Wv"][l] * g1[l], NCT) for l in range(L)])
    wo = np.stack([t8(f["Wo"][l], NCT) for l in range(L)])
    w1 = np.stack([tile_out(f["W1"][l] * g2[l], NF) for l in range(L)])
    # w2: [F, C] -> [4, 128, u, 256] quarter-major
    w2 = np.stack([
        np.ascontiguousarray(
            f["W2"][l].reshape(NF, P, 4, 256).transpose(2, 1, 0, 3))
        for l in range(L)])
    wh = t8(f["Whead"] * f["lnf_g"][:, None], NCT)

    bqc = np.stack([col(b1n[l] @ f["Wq"][l], NCT) for l in range(L)])
    bkc = np.stack([col(b1n[l] @ f["Wk"][l], NCT) for l in range(L)])
    bvr = np.stack([(b1n[l] @ f["Wv"][l])[None] for l in range(L)])
    bor = f["bo"][:, None, :]
    b2r = f["b2"][:, None, :]
    b1c = np.stack([col(b2n[l] @ f["W1"][l] + f["b1"][l], NF)
                    for l in range(L)])
    bh = (f["lnf_b"] @ (f["Whead"] * f["lnf_g"][:, None]) + f["bhead"])[None]

    tokT = t8(f["tok_emb"], NV)
    pos = t8(f["pos_emb"][:T], NT).astype(np.float32)

    common = {
        "wq": wq.astype(_BF), "wk": wk.astype(_BF), "wv": wv.astype(_BF),
        "wo": wo.astype(_BF), "w1": w1.astype(_BF), "w2": w2.astype(_BF),
        "wh": wh.astype(_BF),
        "bqc": bqc, "bkc": bkc, "b1c": b1c,
        "bvr": bvr.astype(_BF), "bor": bor.astype(_BF),
        "b2r": b2r.astype(_BF),
        "bh": bh.astype(np.float32),
        "tokT": tokT.astype(_BF), "pos": pos,
        "ident": np.eye(P, dtype=_BF),
        "mask": np.triu(np.ones((P, P))).astype(_BF),
    }

    idx = f["idx"].astype(np.int64)
    in_maps = []
    for b in range(B):
        oh = (np.arange(V)[:, None] == idx[b][None, :]).astype(np.float32)
        ohT = np.ascontiguousarray(
            oh.reshape(NV, P, T).transpose(1, 0, 2)).astype(_BF)
        m = dict(common)
        m["ohT"] = ohT
        in_maps.append(m)
    return in_maps


def kernel(**inputs):
    if "nc" not in _COMPILED:
        _COMPILED["nc"] = _build_nc()
    nc = _COMPILED["nc"]
    in_maps = _prep_inputs(inputs)
    res = run_bass_kernel_spmd(nc, in_maps, core_ids=list(range(B)))
    outs = []
    for b in range(B):
        o = np.asarray(res.results[b]["out"])  # [128, 8, 256]
        outs.append(o.transpose(1, 0, 2).reshape(T, V))
    return np.stack(outs).astype(np.float32)
